# revision 7
# baseline (speedup 1.0000x reference)
"""Trainium2 Bass kernel for nn_DilatedAttention (B=2, L=2048, D=1024, H=16,
DH=64, HIDDEN=4096, dilation=2, window=512, causal, pre-norm block).

Sharding: sequence-parallel over B*L across 8 cores (512 own rows each) with a
512-row halo for the attention window — no collectives.  Dilation handled by
parity-deinterleaving (even/odd subsequences -> dense causal window of 256).
Matmuls run in float32r (fp32 with 11-bit mantissa) at full PE rate; softmax
denominator comes from a ones-augmented V column; LN gains/biases are folded
into the adjacent weight matrices on the host.
"""
import sys

sys.path.insert(0, "/opt/trn_rl_repo")

import numpy as np

B, L, D = 2, 2048, 1024
H, DH = 16, 64
HIDDEN = 4096
EPS = 1e-5
OWN, HALO = 512, 512
EXT = OWN + HALO
NCORE = 8
PSUB = OWN // 2     # own rows per parity
KSUB = EXT // 2     # ext keys per parity
WIN = 256           # window in subseq coords
SCALE = 1.0 / 8.0   # 1/sqrt(DH)


# ---------------------------------------------------------------- host utils
def _round_f32r(x):
    b = np.ascontiguousarray(x, dtype=np.float32).view(np.uint32)
    low = b & np.uint32(0xFFF)
    base = b & np.uint32(0xFFFFF000)
    lsb = (b >> np.uint32(12)) & np.uint32(1)
    up = (low > 0x800) | ((low == 0x800) & (lsb == 1))
    return (base + (up.astype(np.uint32) << np.uint32(12))).view(np.float32)


def _make_mask(batch_start):
    v = np.arange(KSUB)[:, None]
    u = np.arange(PSUB)[None, :]
    m = (v >= u) & (v <= u + WIN)
    if batch_start:
        m &= v >= HALO // 2
    return np.ascontiguousarray(m.astype(np.float32).reshape(4, 128, PSUB))


# ------------------------------------------------------------- device build
_CACHE = {}


def _split_excess_waits(nc, mybir, budget=1):
    """TPB instructions carry one HW sync-wait slot; hoist excess waits onto
    same-engine InstNoOps inserted just before the instruction."""
    ok = {"InstAllEngineBarrier", "InstEventSemaphore"}
    for f in nc.m.functions:
        for blk in f.blocks:
            out = []
            for ins in blk.instructions:
                si = ins.sync_info
                if (si is not None and type(ins).__name__ not in ok
                        and len(si.on_wait) > budget):
                    waits = list(si.on_wait)
                    for w in waits[:-budget]:
                        out.append(mybir.InstNoOp(
                            name=nc.get_next_instruction_name(),
                            sync_info=mybir.SyncInfo(on_wait=[w], on_update=[]),
                            engine=ins.engine,
                            bass_nofuse=True,
                        ))
                    ins.sync_info = mybir.SyncInfo(
                        on_wait=waits[-budget:], on_update=si.on_update)
                out.append(ins)
            blk.instructions[:] = out


def _build():
    if "nc" in _CACHE:
        return _CACHE["nc"]
    import concourse.bass as bass
    import concourse.mybir as mybir
    import concourse.tile as tile
    from concourse.masks import make_identity

    F32 = mybir.dt.float32
    F32R = mybir.dt.float32r
    AF = mybir.ActivationFunctionType
    OP = mybir.AluOpType

    nc = bass.Bass()
    d_x = nc.declare_dram_parameter("x_ext", [EXT, D], F32, isOutput=False)
    d_wqkv = nc.declare_dram_parameter("wqkv", [16, 128, 8 * 128], F32R, isOutput=False)
    d_wv = nc.declare_dram_parameter("wv", [8, 128, D], F32R, isOutput=False)
    d_wo = nc.declare_dram_parameter("wo", [8, 128, D], F32R, isOutput=False)
    d_wff1 = nc.declare_dram_parameter("wff1", [32, 128, 8 * 128], F32R, isOutput=False)
    d_wff2 = nc.declare_dram_parameter("wff2", [32, 128, D], F32R, isOutput=False)
    d_bqkv = nc.declare_dram_parameter("bqkv", [128, 16], F32, isOutput=False)
    d_bv = nc.declare_dram_parameter("bv", [1, D], F32, isOutput=False)
    d_bo = nc.declare_dram_parameter("bo", [1, D], F32, isOutput=False)
    d_bff1 = nc.declare_dram_parameter("bff1", [128, 32], F32, isOutput=False)
    d_bff2 = nc.declare_dram_parameter("bff2", [1, D], F32, isOutput=False)
    d_mask = nc.declare_dram_parameter("mask", [4, 128, PSUB], F32, isOutput=False)
    d_out = nc.declare_dram_parameter("out", [OWN, D], F32, isOutput=True)

    with tile.TileContext(nc) as tc:
        with tc.tile_pool(name="const", bufs=1) as cst, \
             tc.tile_pool(name="xown", bufs=1) as xop, \
             tc.tile_pool(name="attnT", bufs=1) as atp:

            # ---- constants
            ident = cst.tile([128, 128], F32)
            make_identity(nc, ident)
            eps_sb = cst.tile([128, 1], F32)
            nc.vector.memset(eps_sb, EPS)
            ones16 = cst.tile([128, 16], F32)
            nc.vector.memset(ones16, 1.0)
            onec_f = cst.tile([1, 64], F32)
            nc.vector.memset(onec_f, 1.0)
            ones_col = cst.tile([1, 64], F32R)
            nc.vector.tensor_copy(out=ones_col, in_=onec_f)
            mask_sb = cst.tile([128, 4, PSUB], F32)
            for kc in range(4):
                nc.sync.dma_start(out=mask_sb[:, kc, :], in_=d_mask[kc])
            zeros128 = cst.tile([128, 128], F32)
            nc.vector.memset(zeros128, 0.0)
            bqkv_sb = cst.tile([128, 16], F32)
            nc.sync.dma_start(out=bqkv_sb, in_=d_bqkv[:, :])
            bff1_sb = cst.tile([128, 32], F32)
            nc.sync.dma_start(out=bff1_sb, in_=d_bff1[:, :])
            bv_bc = cst.tile([128, D], F32)
            nc.sync.dma_start(out=bv_bc, in_=d_bv[:, :].to_broadcast([128, D]))
            bo_bc = cst.tile([128, D], F32)
            nc.sync.dma_start(out=bo_bc, in_=d_bo[:, :].to_broadcast([128, D]))
            bff2_bc = cst.tile([128, D], F32)
            nc.sync.dma_start(out=bff2_bc, in_=d_bff2[:, :].to_broadcast([128, D]))

            # x own rows stay resident for residual 1
            x_own = [xop.tile([128, D], F32, tag=f"xo{rc}", name=f"xo{rc}") for rc in range(4)]
            attn_T = [atp.tile([128, OWN], F32R, tag=f"at{fc}", name=f"at{fc}") for fc in range(8)]

            with tc.tile_pool(name="qkvout", bufs=1) as qkp:
                Q_T = [qkp.tile([128, 2, PSUB], F32R, tag=f"q{fc}", name=f"q{fc}") for fc in range(8)]
                K_T = [qkp.tile([128, 2, KSUB], F32R, tag=f"k{fc}", name=f"k{fc}") for fc in range(8)]
                V_sb = [[qkp.tile([128, H, 66], F32R, tag=f"v{p}{kc}", name=f"v{p}{kc}") for kc in range(4)]
                        for p in range(2)]

                # ================= phase A: LN1 + transpose -> hT ==========
                with tc.tile_pool(name="hT", bufs=1) as htp:
                    hT = [htp.tile([128, 2, KSUB], F32R, tag=f"h{dc}", name=f"h{dc}") for dc in range(8)]
                    with tc.tile_pool(name="lntmp", bufs=3) as lnt, \
                         tc.tile_pool(name="psA", bufs=4, space="PSUM") as psA:
                        for rc in range(8):
                            if rc >= 4:
                                x_sb = x_own[rc - 4]
                            else:
                                x_sb = lnt.tile([128, D], F32, tag="xt", name="xt")
                            nc.sync.dma_start(out=x_sb, in_=d_x[rc * 128:(rc + 1) * 128, :])
                            stats = lnt.tile([128, 2, 6], F32, tag="st", name="st")
                            x3 = x_sb.rearrange("p (s d) -> p s d", s=2)
                            nc.vector.bn_stats(out=stats[:, 0, :], in_=x3[:, 0, :])
                            nc.vector.bn_stats(out=stats[:, 1, :], in_=x3[:, 1, :])
                            mv = lnt.tile([128, 2], F32, tag="mv", name="mv")
                            nc.vector.bn_aggr(out=mv, in_=stats)
                            sd = lnt.tile([128, 1], F32, tag="sd", name="sd")
                            nc.scalar.activation(out=sd, in_=mv[:, 1:2], func=AF.Sqrt,
                                                 bias=eps_sb, scale=1.0)
                            rstd = lnt.tile([128, 1], F32, tag="rs", name="rs")
                            nc.vector.reciprocal(out=rstd, in_=sd)
                            h_sb = lnt.tile([128, D], F32, tag="hh", name="hh")
                            nc.vector.tensor_scalar(out=h_sb, in0=x_sb,
                                                    scalar1=mv[:, 0:1], scalar2=rstd,
                                                    op0=OP.subtract, op1=OP.mult)
                            for dc in range(8):
                                pt = psA.tile([128, 128], F32, tag="pt", name="pt")
                                nc.tensor.transpose(pt, h_sb[:, dc * 128:(dc + 1) * 128], ident)
                                nc.scalar.activation(
                                    out=hT[dc][:, :, rc * 64:(rc + 1) * 64],
                                    in_=pt.rearrange("d (j two) -> d two j", two=2),
                                    func=AF.Identity)

                    # ================= phase B: QKV projections ============
                    with tc.tile_pool(name="wq", bufs=3) as wqp, \
                         tc.tile_pool(name="psQ", bufs=2, space="PSUM") as psQ, \
                         tc.tile_pool(name="psK", bufs=2, space="PSUM") as psK:
                        for fc in range(16):  # 0-7 = Q feats, 8-15 = K feats
                            w_sb = wqp.tile([128, 8, 128], F32R, tag="wq", name="wq")
                            nc.sync.dma_start(out=w_sb, in_=d_wqkv[fc].rearrange(
                                "p (dc f) -> p dc f", dc=8))
                            for p in range(2):
                                if fc < 8:
                                    ps = psQ.tile([128, PSUB], F32, tag="q", name="q")
                                    for dc in range(8):
                                        nc.tensor.matmul(ps, w_sb[:, dc, :],
                                                         hT[dc][:, p, 256:512],
                                                         start=(dc == 0), stop=(dc == 7))
                                    nc.scalar.activation(out=Q_T[fc][:, p, :], in_=ps,
                                                         func=AF.Identity,
                                                         bias=bqkv_sb[:, fc:fc + 1])
                                else:
                                    ps = psK.tile([128, KSUB], F32, tag="k", name="k")
                                    for dc in range(8):
                                        nc.tensor.matmul(ps, w_sb[:, dc, :],
                                                         hT[dc][:, p, 0:512],
                                                         start=(dc == 0), stop=(dc == 7))
                                    nc.scalar.activation(out=K_T[fc - 8][:, p, :], in_=ps,
                                                         func=AF.Identity,
                                                         bias=bqkv_sb[:, fc:fc + 1])

                    with tc.tile_pool(name="wv", bufs=1) as wvp, \
                         tc.tile_pool(name="psV", bufs=2, space="PSUM") as psV:
                        wv_sb = [wvp.tile([128, D], F32R, tag=f"wv{dc}", name=f"wv{dc}") for dc in range(8)]
                        for dc in range(8):
                            nc.sync.dma_start(out=wv_sb[dc], in_=d_wv[dc])
                        for p in range(2):
                            for kc in range(4):
                                for nh in range(2):
                                    ps = psV.tile([128, 512], F32, tag="v", name="v")
                                    for dc in range(8):
                                        nc.tensor.matmul(
                                            ps, hT[dc][:, p, kc * 128:(kc + 1) * 128],
                                            wv_sb[dc][:, nh * 512:(nh + 1) * 512],
                                            start=(dc == 0), stop=(dc == 7))
                                    nc.vector.tensor_tensor(
                                        out=V_sb[p][kc][:, nh * 8:(nh + 1) * 8, 0:64],
                                        in0=ps.rearrange("k (h d) -> k h d", d=64),
                                        in1=bv_bc[:, nh * 512:(nh + 1) * 512].rearrange(
                                            "k (h d) -> k h d", d=64),
                                        op=OP.add)
                                nc.vector.tensor_copy(
                                    out=V_sb[p][kc][:, :, 64:65],
                                    in_=ones16.rearrange("p (h o) -> p h o", o=1))
                # hT freed here

                # ================= phase C: attention ======================
                with tc.tile_pool(name="pexp", bufs=10) as pep, \
                     tc.tile_pool(name="pmsk", bufs=10) as pmp, \
                     tc.tile_pool(name="tiny", bufs=4) as tnp, \
                     tc.tile_pool(name="rbp", bufs=4) as rbp, \
                     tc.tile_pool(name="psS", bufs=4, space="PSUM") as psS, \
                     tc.tile_pool(name="psO", bufs=2, space="PSUM") as psO, \
                     tc.tile_pool(name="psB", bufs=2, space="PSUM") as psB:
                    for hh in range(H):
                        fc, kb = hh // 2, (hh % 2) * 64
                        for p in range(2):
                            pm = []
                            for kc in range(4):
                                s_ps = psS.tile([128, PSUB], F32, tag="s", name="s")
                                nc.tensor.matmul(
                                    s_ps,
                                    K_T[fc][kb:kb + 64, p, kc * 128:(kc + 1) * 128],
                                    Q_T[fc][kb:kb + 64, p, :],
                                    start=True, stop=True)
                                pe = pep.tile([128, PSUB], F32, tag="pe", name="pe")
                                pmt = pmp.tile([128, PSUB], F32R, tag="pm", name="pm")
                                # quarter-tiles fully outside the band: skip
                                if kc == 0:
                                    lo, hi = 0, 128
                                    nc.vector.tensor_copy(out=pmt[:, 128:256], in_=zeros128)
                                elif kc == 3:
                                    lo, hi = 128, 256
                                    nc.vector.tensor_copy(out=pmt[:, 0:128], in_=zeros128)
                                else:
                                    lo, hi = 0, 256
                                nc.scalar.activation(out=pe[:, lo:hi], in_=s_ps[:, lo:hi],
                                                     func=AF.Exp)
                                nc.vector.tensor_tensor(
                                    out=pmt[:, lo:hi], in0=pe[:, lo:hi],
                                    in1=mask_sb[:, kc, lo:hi], op=OP.mult)
                                pm.append(pmt)
                            o_ps = psO.tile([65, PSUB], F32, tag="o", name="o")
                            for kc in range(4):
                                nc.tensor.matmul(o_ps, V_sb[p][kc][:, hh, 0:65], pm[kc],
                                                 start=(kc == 0), stop=(kc == 3))
                            rec = tnp.tile([1, PSUB], F32, tag="rc", name="rc")
                            nc.vector.reciprocal(out=rec, in_=o_ps[64:65, :])
                            r_row = tnp.tile([1, PSUB], F32R, tag="rr", name="rr")
                            nc.vector.tensor_copy(out=r_row, in_=rec)
                            b_ps = psB.tile([64, PSUB], F32, tag="b", name="b")
                            nc.tensor.matmul(b_ps, ones_col, r_row, start=True, stop=True)
                            rb = rbp.tile([64, PSUB], F32, tag="rb", name="rb")
                            nc.scalar.activation(out=rb, in_=b_ps, func=AF.Copy)
                            nc.vector.scalar_tensor_tensor(
                                out=attn_T[fc][kb:kb + 64].rearrange(
                                    "d (u two) -> d two u", two=2)[:, p, :],
                                in0=o_ps[0:64, :], scalar=1.0, in1=rb,
                                op0=OP.mult, op1=OP.mult)
            # Q/K/V freed here

            with tc.tile_pool(name="res1", bufs=1) as rp:
                res1 = [rp.tile([128, D], F32, tag=f"r{rc}", name=f"r{rc}") for rc in range(4)]

                # ================= phase D: out-proj + residual 1 ==========
                with tc.tile_pool(name="wo", bufs=1) as wop, \
                     tc.tile_pool(name="tD", bufs=3) as tdp, \
                     tc.tile_pool(name="psD", bufs=3, space="PSUM") as psD:
                    wo_sb = [wop.tile([128, D], F32R, tag=f"wo{fc}", name=f"wo{fc}") for fc in range(8)]
                    for fc in range(8):
                        nc.sync.dma_start(out=wo_sb[fc], in_=d_wo[fc])
                    for rc in range(4):
                        for nh in range(2):
                            ps = psD.tile([128, 512], F32, tag="d", name="d")
                            for fc in range(8):
                                nc.tensor.matmul(ps, attn_T[fc][:, rc * 128:(rc + 1) * 128],
                                                 wo_sb[fc][:, nh * 512:(nh + 1) * 512],
                                                 start=(fc == 0), stop=(fc == 7))
                            t1 = tdp.tile([128, 512], F32, tag="t1", name="t1")
                            nc.vector.tensor_tensor(out=t1, in0=ps,
                                                    in1=x_own[rc][:, nh * 512:(nh + 1) * 512],
                                                    op=OP.add)
                            nc.vector.tensor_tensor(
                                out=res1[rc][:, nh * 512:(nh + 1) * 512], in0=t1,
                                in1=bo_bc[:, nh * 512:(nh + 1) * 512], op=OP.add)

                # ================= phase E: LN2 + transpose -> h2T =========
                with tc.tile_pool(name="h2T", bufs=1) as h2p:
                    h2T = [h2p.tile([128, OWN], F32R, tag=f"h2{dc}", name=f"h2{dc}") for dc in range(8)]
                    with tc.tile_pool(name="lnt2", bufs=3) as ln2, \
                         tc.tile_pool(name="psE", bufs=4, space="PSUM") as psE:
                        for rc in range(4):
                            stats = ln2.tile([128, 2, 6], F32, tag="st", name="st")
                            r3 = res1[rc].rearrange("p (s d) -> p s d", s=2)
                            nc.vector.bn_stats(out=stats[:, 0, :], in_=r3[:, 0, :])
                            nc.vector.bn_stats(out=stats[:, 1, :], in_=r3[:, 1, :])
                            mv = ln2.tile([128, 2], F32, tag="mv", name="mv")
                            nc.vector.bn_aggr(out=mv, in_=stats)
                            sd = ln2.tile([128, 1], F32, tag="sd", name="sd")
                            nc.scalar.activation(out=sd, in_=mv[:, 1:2], func=AF.Sqrt,
                                                 bias=eps_sb, scale=1.0)
                            rstd = ln2.tile([128, 1], F32, tag="rs", name="rs")
                            nc.vector.reciprocal(out=rstd, in_=sd)
                            h2 = ln2.tile([128, D], F32, tag="h2", name="h2")
                            nc.vector.tensor_scalar(out=h2, in0=res1[rc],
                                                    scalar1=mv[:, 0:1], scalar2=rstd,
                                                    op0=OP.subtract, op1=OP.mult)
                            for dc in range(8):
                                pt = psE.tile([128, 128], F32, tag="pt", name="pt")
                                nc.tensor.transpose(pt, h2[:, dc * 128:(dc + 1) * 128], ident)
                                nc.scalar.activation(
                                    out=h2T[dc][:, rc * 128:(rc + 1) * 128], in_=pt,
                                    func=AF.Identity)

                    # ================= phase F: FF1 + gelu =================
                    with tc.tile_pool(name="gelu", bufs=1) as gp:
                        gelu_T = [gp.tile([128, OWN], F32R, tag=f"g{hc}", name=f"g{hc}") for hc in range(32)]
                        with tc.tile_pool(name="w1", bufs=3) as w1p, \
                             tc.tile_pool(name="psF", bufs=3, space="PSUM") as psF:
                            for hc in range(32):
                                w_sb = w1p.tile([128, 8, 128], F32R, tag="w1", name="w1")
                                nc.sync.dma_start(out=w_sb, in_=d_wff1[hc].rearrange(
                                    "p (dc f) -> p dc f", dc=8))
                                ps = psF.tile([128, OWN], F32, tag="f", name="f")
                                for dc in range(8):
                                    nc.tensor.matmul(ps, w_sb[:, dc, :], h2T[dc],
                                                     start=(dc == 0), stop=(dc == 7))
                                nc.scalar.activation(out=gelu_T[hc], in_=ps, func=AF.Gelu,
                                                     bias=bff1_sb[:, hc:hc + 1], scale=1.0)

                        # ============= phase G: FF2 + residual 2 + store ===
                        with tc.tile_pool(name="w2", bufs=4) as w2p, \
                             tc.tile_pool(name="outp", bufs=1) as otp, \
                             tc.tile_pool(name="psG", bufs=1, space="PSUM") as psG:
                            gps = [psG.tile([128, 512], F32, tag=f"G{i}", name=f"G{i}") for i in range(8)]
                            for hc in range(32):
                                w_sb = w2p.tile([128, D], F32R, tag="w2", name="w2")
                                nc.sync.dma_start(out=w_sb, in_=d_wff2[hc])
                                for rc in range(4):
                                    for nh in range(2):
                                        nc.tensor.matmul(
                                            gps[rc * 2 + nh],
                                            gelu_T[hc][:, rc * 128:(rc + 1) * 128],
                                            w_sb[:, nh * 512:(nh + 1) * 512],
                                            start=(hc == 0), stop=(hc == 31))
                            for rc in range(4):
                                o_sb = otp.tile([128, D], F32, tag=f"os{rc}", name=f"os{rc}")
                                for nh in range(2):
                                    t1 = otp.tile([128, 512], F32, tag="t2", name="t2", bufs=2)
                                    nc.vector.tensor_tensor(
                                        out=t1, in0=gps[rc * 2 + nh],
                                        in1=res1[rc][:, nh * 512:(nh + 1) * 512], op=OP.add)
                                    nc.vector.tensor_tensor(
                                        out=o_sb[:, nh * 512:(nh + 1) * 512], in0=t1,
                                        in1=bff2_bc[:, nh * 512:(nh + 1) * 512], op=OP.add)
                                nc.sync.dma_start(out=d_out[rc * 128:(rc + 1) * 128, :],
                                                  in_=o_sb)

    _split_excess_waits(nc, mybir)
    _CACHE["nc"] = nc
    return nc


# ------------------------------------------------------------- host wrapper
def _prep(inputs):
    f32 = np.float32
    x = np.asarray(inputs["x"], f32)
    g1 = np.asarray(inputs["ln1_g"], f32)
    b1 = np.asarray(inputs["ln1_b"], f32)
    wqkv = np.asarray(inputs["w_qkv"], f32)
    bqkv = np.asarray(inputs["b_qkv"], f32)
    wo = np.asarray(inputs["w_o"], f32)
    bo = np.asarray(inputs["b_o"], f32)
    g2 = np.asarray(inputs["ln2_g"], f32)
    b2 = np.asarray(inputs["ln2_b"], f32)
    wff1 = np.asarray(inputs["w_ff1"], f32)
    bff1 = np.asarray(inputs["b_ff1"], f32)
    wff2 = np.asarray(inputs["w_ff2"], f32)
    bff2 = np.asarray(inputs["b_ff2"], f32)

    wqkv_p = (wqkv * g1[None, :]).astype(f32)
    bqkv_p = (wqkv @ b1 + bqkv).astype(f32)
    wqkv_p[:D] *= SCALE
    bqkv_p = bqkv_p.copy()
    bqkv_p[:D] *= SCALE
    wff1_p = (wff1 * g2[None, :]).astype(f32)
    bff1_p = (wff1 @ b2 + bff1).astype(f32)

    wt = _round_f32r(wqkv_p.T)                       # [D, 3D] = W'.T
    # q/k feat blocks, packed [fc][p][dc*128+f]
    wqk = np.stack([wt[:, fc * 128:(fc + 1) * 128]   # [1024, 128]
                    .reshape(8, 128, 128).transpose(1, 0, 2).reshape(128, 1024)
                    for fc in range(16)])             # [16, 128, 1024]
    wv = np.ascontiguousarray(wt[:, 2 * D:].reshape(8, 128, D))
    wo_t = _round_f32r(np.ascontiguousarray(wo.T.reshape(8, 128, D)))
    w1t = _round_f32r(wff1_p.T)                      # [D, HIDDEN]
    w1 = np.stack([w1t[:, hc * 128:(hc + 1) * 128]
                   .reshape(8, 128, 128).transpose(1, 0, 2).reshape(128, 1024)
                   for hc in range(32)])              # [32, 128, 1024]
    w2 = _round_f32r(np.ascontiguousarray(wff2.T.reshape(32, 128, D)))

    bqkv_c = np.ascontiguousarray(bqkv_p[:2 * D].reshape(16, 128).T)   # [128, 16]
    bff1_c = np.ascontiguousarray(bff1_p.reshape(32, 128).T)           # [128, 32]
    bv_c = np.ascontiguousarray(bqkv_p[2 * D:].reshape(1, D))
    bo_c = np.ascontiguousarray(bo.reshape(1, D))
    bff2_c = np.ascontiguousarray(bff2.reshape(1, D))

    mask_mid = _make_mask(False)
    mask_start = _make_mask(True)

    shared = {
        "wqkv": np.ascontiguousarray(wqk), "wv": wv, "wo": wo_t,
        "wff1": np.ascontiguousarray(w1), "wff2": w2,
        "bqkv": bqkv_c, "bv": bv_c, "bo": bo_c, "bff1": bff1_c, "bff2": bff2_c,
    }
    in_maps = []
    for c in range(NCORE):
        b, s = c // 4, c % 4
        S = s * OWN
        x_ext = np.zeros((EXT, D), f32)
        lo = S - HALO
        x_ext[max(0, -lo):] = x[b, max(lo, 0):S + OWN]
        m = dict(shared)
        m["x_ext"] = x_ext
        m["mask"] = mask_start if s == 0 else mask_mid
        in_maps.append(m)
    return in_maps


def _run(inputs, trace=False):
    from concourse.bass_utils import run_bass_kernel_spmd
    nc = _build()
    in_maps = _prep(inputs)
    res = run_bass_kernel_spmd(nc, in_maps, core_ids=list(range(NCORE)),
                               trace=trace)
    out = np.zeros((B, L, D), np.float32)
    for c in range(NCORE):
        b, s = c // 4, c % 4
        out[b, s * OWN:(s + 1) * OWN] = res.results[c]["out"]
    return out, res


def kernel(**inputs):
    out, _ = _run(inputs)
    return out
